# revision 7
# baseline (speedup 1.0000x reference)
"""DeepseekV4 indexer kernel for 8 trn2 NeuronCores (Bass/Tile).

Strategy (token-sharded, two bass launches):
  - Tokens are split into 16 tiles of 128; core i owns tiles (i, 15-i) so the
    causally-pruned top-k work is balanced across cores.
  - Launch 1 (per core): fused W_fused/Wproj GEMM over the core's 264-token
    halo'd shard -> compressor softmax -> RMSNorm -> RoPE -> compressed K for
    its 64 compressed positions, plus per-token head weights.  Outputs are
    tiny ([64,128] K + [256,64] wts per core).
  - Host: concatenates the per-core K shards into the full [512,128] K
    (the "all-gather"; collectives don't load on this runtime).
  - Launch 2 (per core): q = qr @ Wq.T (fp32), RoPE, qk against full K,
    relu * wts accumulation over 64 heads, causal mask, and iterated
    top-8 (max / max_index / match_replace) producing the top-256 indices
    in descending-score order.  All GEMMs run in exact fp32 so the ordering
    matches the fp32 reference up to fp32 rounding noise.

kernel(**inputs) takes the FULL unsharded inputs and returns [2048,256] int32.
"""
import sys
sys.path.insert(0, '/opt/trn_rl_repo')

from contextlib import ExitStack

import numpy as np

import concourse.bass as bass
import concourse.bacc as bacc
import concourse.tile as tile
from concourse import mybir
from concourse.bass_utils import run_bass_kernel_spmd
from concourse.masks import make_identity

T, HID, QR_DIM, H, D, TOPK, R = 2048, 7168, 1536, 64, 128, 256, 4
C = T // R
NC = 8
EPS = 1e-6
F32 = mybir.dt.float32
I32 = mybir.dt.int32
U32 = mybir.dt.uint32
WTS_SCALE = float(H ** -0.5) * float(D ** -0.5)  # folds q's D**-0.5 into wts
NEG = -1e30

PAIRS = [(i, 15 - i) for i in range(NC)]  # token tiles owned by core i

_cache = {}


# --------------------------------------------------------------------------
# launch 1: compressor -> per-core compressed K (64 rows) + head weights
# --------------------------------------------------------------------------
def _build_l1():
    nc = bacc.Bacc()
    hidden = nc.declare_dram_parameter("hidden", [264, HID], F32, isOutput=False)
    wcomb = nc.declare_dram_parameter("wcomb", [HID, 576], F32, isOutput=False)
    ape = nc.declare_dram_parameter("ape", [8, D], F32, isOutput=False)
    rmsw = nc.declare_dram_parameter("rmsw", [D], F32, isOutput=False)
    cs_k = nc.declare_dram_parameter("cs_k", [64, D], F32, isOutput=False)
    haloflag = nc.declare_dram_parameter("haloflag", [2], F32, isOutput=False)
    k_loc = nc.declare_dram_parameter("k_loc", [64, D], F32, isOutput=True)
    wts_own = nc.declare_dram_parameter("wts_own", [256, H], F32, isOutput=True)

    with tile.TileContext(nc) as tc, ExitStack() as ctx:
        const = ctx.enter_context(tc.tile_pool(name="const", bufs=1))
        big = ctx.enter_context(tc.tile_pool(name="big", bufs=1))
        work = ctx.enter_context(tc.tile_pool(name="work", bufs=2))

        ident = const.tile([128, 128], F32)
        make_identity(nc, ident)

        def tp(ps_out, in_sb):
            p = in_sb.shape[0]
            nc.tensor.transpose(ps_out, in_sb, ident[:p, :p])

        # ---- hiddenT via PE transposes, split so the GEMM can start on
        # tokens 0:128 while tokens 128:264 are still being transposed ----
        hidT_a = big.tile([128, 56, 128], F32)
        hidT_b = big.tile([128, 56, 136], F32)
        kvt = []
        wts_sb = work.tile([64, 264], F32, tag="wts_sb")
        with tc.tile_pool(name="stg", bufs=2) as stg, \
             tc.tile_pool(name="tpsA", bufs=2, space="PSUM") as tpsA, \
             tc.tile_pool(name="wstg", bufs=3) as wstg, \
             tc.tile_pool(name="gps", bufs=1, space="PSUM") as gps:
            kvps = [gps.tile([128, 264], F32, tag=f"kvps{m}", name=f"kvps{m}") for m in range(4)]
            wtsps = gps.tile([64, 264], F32, tag="wtsps")
            for (t0, rows, dst, c0) in [(0, 128, hidT_a, 0),
                                        (128, 128, hidT_b, 0),
                                        (256, 8, hidT_b, 128)]:
                stage = stg.tile([128, HID], F32, tag="stage")
                nc.sync.dma_start(out=stage[:rows, :], in_=hidden[t0:t0 + rows, :])
                for kg in range(14):
                    ps = tpsA.tile([128, 512], F32, tag="tp")
                    for u in range(4):
                        kc = kg * 4 + u
                        tp(ps[:, u * 128:u * 128 + rows],
                           stage[:rows, kc * 128:(kc + 1) * 128])
                    sv = ps.rearrange("p (u x) -> p u x", x=128)[:, :, :rows]
                    nc.scalar.copy(dst[:, kg * 4:kg * 4 + 4, c0:c0 + rows], sv)
            for phase, (rhs_of, csl) in enumerate(
                    [(lambda kc: hidT_a[:, kc, :], slice(0, 128)),
                     (lambda kc: hidT_b[:, kc, :], slice(128, 264))]):
                for kc in range(56):
                    wt = wstg.tile([128, 576], F32, tag="wcomb")
                    nc.sync.dma_start(out=wt,
                                      in_=wcomb[kc * 128:(kc + 1) * 128, :])
                    for m in range(4):
                        nc.tensor.matmul(kvps[m][:, csl],
                                         wt[:, m * 128:(m + 1) * 128],
                                         rhs_of(kc), start=(kc == 0),
                                         stop=(kc == 55))
                    nc.tensor.matmul(wtsps[:, csl], wt[:, 512:576], rhs_of(kc),
                                     start=(kc == 0), stop=(kc == 55))
            for m in range(4):
                t = work.tile([128, 264], F32, tag=f"kvt{m}")
                nc.scalar.copy(t, kvps[m])
                kvt.append(t)
            nc.scalar.mul(wts_sb, wtsps, WTS_SCALE)
        kv_old, kv_new, sc_old, sc_new = kvt

        with tc.tile_pool(name="tpsB", bufs=2, space="PSUM") as tpsB:
            # wts -> [t, h] and out
            for s in range(2):
                ps = tpsB.tile([128, 64], F32, tag="wtp")
                tp(ps, wts_sb[:, 4 + 132 * s:132 + 132 * s])
                ob = work.tile([128, 64], F32, tag="wob")
                nc.scalar.copy(ob, ps)
                nc.sync.dma_start(out=wts_own[128 * s:128 * (s + 1), :], in_=ob)

            # ape transposed + replicated [128, 32, 8]
            ape_st = work.tile([8, D], F32, tag="ape_st")
            nc.sync.dma_start(out=ape_st, in_=ape[:])
            aps = tpsB.tile([128, 8], F32, tag="apetp")
            tp(aps, ape_st)
            apeT = const.tile([128, 8], F32)
            nc.scalar.copy(apeT, aps)
            ape_rep = const.tile([128, 32, 8], F32)
            for g in range(32):
                nc.vector.tensor_copy(ape_rep[:, g, :], apeT)

            # rms weight replicated [32, 128]
            rms_rep = const.tile([32, D], F32)
            nc.sync.dma_start(out=rms_rep, in_=bass.AP(
                tensor=rmsw, offset=0, ap=[[0, 32], [1, D]]))

            cs_st = []
            for s in range(2):
                cst = const.tile([32, D], F32, tag=f"cs{s}", name=f"cs{s}")
                nc.sync.dma_start(out=cst, in_=cs_k[32 * s:32 * s + 32, :])
                cs_st.append(cst)

            hf = []
            for s in range(2):
                h = const.tile([128, 1], F32, tag=f"hf{s}")
                nc.sync.dma_start(out=h, in_=bass.AP(
                    tensor=haloflag, offset=s, ap=[[0, 128], [1, 1]]))
                hf.append(h)

            for s in range(2):
                o = 132 * s
                gates = work.tile([128, 32, 8], F32, tag="gates")
                so_v = sc_old[:, o:o + 128].rearrange("p (g x) -> p g x", x=4)
                sn_v = sc_new[:, o + 4:o + 132].rearrange("p (g x) -> p g x", x=4)
                ko_v = kv_old[:, o:o + 128].rearrange("p (g x) -> p g x", x=4)
                kn_v = kv_new[:, o + 4:o + 132].rearrange("p (g x) -> p g x", x=4)
                nc.vector.tensor_add(gates[:, :, 0:4], so_v, ape_rep[:, :, 0:4])
                nc.vector.tensor_add(gates[:, :, 4:8], sn_v, ape_rep[:, :, 4:8])
                # first group's old slots += -1e30 when strip starts at t=0
                nc.vector.tensor_scalar(gates[:, 0, 0:4], gates[:, 0, 0:4],
                                        hf[s], None, op0=mybir.AluOpType.add)
                gmax = work.tile([128, 32], F32, tag="gmax")
                nc.vector.reduce_max(gmax, gates, axis=mybir.AxisListType.X)
                nc.vector.tensor_sub(gates, gates,
                                     gmax.to_broadcast([128, 32, 8]))
                ex = work.tile([128, 32, 8], F32, tag="ex")
                nc.scalar.activation(ex, gates, mybir.ActivationFunctionType.Exp)
                den = work.tile([128, 32], F32, tag="den")
                nc.vector.reduce_sum(den, ex, axis=mybir.AxisListType.X)
                rec = work.tile([128, 32], F32, tag="rec")
                nc.vector.reciprocal(rec, den)
                w8 = work.tile([128, 32, 8], F32, tag="w8")
                nc.vector.tensor_mul(w8, ex, rec.to_broadcast([128, 32, 8]))
                prod = work.tile([128, 32, 8], F32, tag="prod")
                nc.vector.tensor_mul(prod[:, :, 0:4], w8[:, :, 0:4], ko_v)
                nc.vector.tensor_mul(prod[:, :, 4:8], w8[:, :, 4:8], kn_v)
                comp = work.tile([128, 32], F32, tag="comp")
                nc.vector.reduce_sum(comp, prod, axis=mybir.AxisListType.X)

                cps = tpsB.tile([32, 128], F32, tag="ctp")
                tp(cps, comp)
                compT = work.tile([32, D], F32, tag="compT")
                nc.scalar.copy(compT, cps)

                # RMSNorm over d
                sq = work.tile([32, D], F32, tag="sq")
                nc.vector.tensor_mul(sq, compT, compT)
                ssum = work.tile([32, 1], F32, tag="ssum")
                nc.vector.reduce_sum(ssum, sq, axis=mybir.AxisListType.X)
                nc.vector.tensor_scalar(ssum, ssum, 1.0 / D, EPS,
                                        op0=mybir.AluOpType.mult,
                                        op1=mybir.AluOpType.add)
                rt = work.tile([32, 1], F32, tag="rt")
                nc.scalar.sqrt(rt, ssum)
                rs = work.tile([32, 1], F32, tag="rs")
                nc.vector.reciprocal(rs, rt)
                nc.vector.tensor_scalar(compT, compT, rs, None,
                                        op0=mybir.AluOpType.mult)
                nc.vector.tensor_mul(compT, compT, rms_rep)

                # RoPE at compressed positions (all tiles at base partition 0)
                co = cs_st[s][:, 0:64]
                si = cs_st[s][:, 64:128]
                x1 = compT[:, 0:64]
                x2 = compT[:, 64:128]
                tmp = work.tile([32, D], F32, tag="ktmp")
                kx = work.tile([32, D], F32, tag="kx")
                nc.vector.tensor_mul(kx[:, 0:64], x1, co)
                nc.vector.tensor_mul(tmp[:, 0:64], x2, si)
                nc.vector.tensor_sub(kx[:, 0:64], kx[:, 0:64], tmp[:, 0:64])
                nc.vector.tensor_mul(kx[:, 64:128], x2, co)
                nc.vector.tensor_mul(tmp[:, 64:128], x1, si)
                nc.vector.tensor_add(kx[:, 64:128], kx[:, 64:128],
                                     tmp[:, 64:128])
                nc.sync.dma_start(out=k_loc[32 * s:32 * s + 32, :], in_=kx)

    nc.finalize()
    return nc


# --------------------------------------------------------------------------
# launch 2: q GEMM + RoPE + qk + score assembly + mask + top-k
# --------------------------------------------------------------------------
def _build_l2():
    nc = bacc.Bacc()
    qr_sh = nc.declare_dram_parameter("qr_sh", [256, QR_DIM], F32, isOutput=False)
    wq = nc.declare_dram_parameter("wq", [H, 128, 12, 128], F32, isOutput=False)
    cs_own = nc.declare_dram_parameter("cs_own", [256, D], F32, isOutput=False)
    k_full = nc.declare_dram_parameter("k_full", [C, D], F32, isOutput=False)
    wts_own = nc.declare_dram_parameter("wts_own", [256, H], F32, isOutput=False)
    posm3 = nc.declare_dram_parameter("posm3", [256], F32, isOutput=False)
    out_idx = nc.declare_dram_parameter("out_idx", [256, TOPK], I32, isOutput=True)

    WIDTHS = (256, 512)  # candidate widths for (low tile j<=7, high tile)

    with tile.TileContext(nc) as tc, ExitStack() as ctx:
        const = ctx.enter_context(tc.tile_pool(name="const", bufs=1))
        work = ctx.enter_context(tc.tile_pool(name="work", bufs=2))
        tk = ctx.enter_context(tc.tile_pool(name="tk", bufs=2))

        ident = const.tile([128, 128], F32)
        make_identity(nc, ident)

        def tp(ps_out, in_sb):
            p = in_sb.shape[0]
            nc.tensor.transpose(ps_out, in_sb, ident[:p, :p])

        qrT = const.tile([128, 12, 256], F32)
        csT = const.tile([128, 256], F32)
        kT = const.tile([128, C], F32)
        with tc.tile_pool(name="stg", bufs=2) as stg, \
             tc.tile_pool(name="tps", bufs=2, space="PSUM") as tps:
            for tt in range(2):
                stage = stg.tile([128, QR_DIM], F32, tag="qstage")
                nc.sync.dma_start(out=stage,
                                  in_=qr_sh[tt * 128:(tt + 1) * 128, :])
                for kg in range(3):
                    ps = tps.tile([128, 512], F32, tag="tp")
                    for u in range(4):
                        kc = kg * 4 + u
                        tp(ps[:, u * 128:(u + 1) * 128],
                           stage[:, kc * 128:(kc + 1) * 128])
                    nc.scalar.copy(
                        qrT[:, kg * 4:kg * 4 + 4, tt * 128:(tt + 1) * 128],
                        ps.rearrange("p (u x) -> p u x", x=128))
            for tt in range(2):
                stage = stg.tile([128, D], F32, tag="cstage")
                nc.sync.dma_start(out=stage,
                                  in_=cs_own[tt * 128:(tt + 1) * 128, :])
                ps = tps.tile([128, 512], F32, tag="tp")
                tp(ps[:, :128], stage)
                nc.scalar.copy(csT[:, tt * 128:(tt + 1) * 128], ps[:, :128])
            kstage = const.tile([128, 4, D], F32)
            nc.sync.dma_start(out=kstage,
                              in_=k_full[:].rearrange("(a p) d -> p a d", p=128))
            for a in range(4):
                ps = tps.tile([128, 512], F32, tag="tp")
                tp(ps[:, :128], kstage[:, a, :])
                nc.scalar.copy(kT[:, a * 128:(a + 1) * 128], ps[:, :128])

        # cc = [cos;cos], ss = [-sin;sin] (partition moves via DMA only)
        cc = const.tile([128, 256], F32)
        ss = const.tile([128, 256], F32)
        nc.sync.dma_start(out=cc[0:64, :], in_=csT[0:64, :])
        nc.sync.dma_start(out=cc[64:128, :], in_=csT[0:64, :])
        nc.sync.dma_start(out=ss[0:64, :], in_=csT[64:128, :])
        nc.sync.dma_start(out=ss[64:128, :], in_=csT[64:128, :])
        nc.vector.tensor_scalar(ss[0:64, :], ss[0:64, :], -1.0, None,
                                op0=mybir.AluOpType.mult)

        wts_sb, pos_sb = [], []
        for tt in range(2):
            w = const.tile([128, H], F32, tag=f"wts{tt}")
            nc.sync.dma_start(out=w, in_=wts_own[tt * 128:(tt + 1) * 128, :])
            wts_sb.append(w)
            p = const.tile([128, 1], F32, tag=f"pos{tt}")
            nc.sync.dma_start(out=p, in_=posm3[tt * 128:(tt + 1) * 128])
            pos_sb.append(p)

        c4p = const.tile([128, C], F32)
        nc.gpsimd.iota(c4p, pattern=[[4, C]], base=0, channel_multiplier=0,
                       allow_small_or_imprecise_dtypes=True)
        c4f = const.tile([128, C], F32)
        nc.vector.tensor_scalar(c4f, c4p, -1.0, None, op0=mybir.AluOpType.mult)
        negs = const.tile([128, C], F32)
        nc.vector.memset(negs, NEG)
        neg1 = const.tile([128, TOPK], I32)
        nc.vector.memset(neg1, -1)

        acc = [const.tile([128, C], F32, tag=f"acc{tt}", name=f"acc{tt}") for tt in range(2)]

        with tc.tile_pool(name="wqp", bufs=3) as wqp, \
             tc.tile_pool(name="qro_p", bufs=2) as qro_p, \
             tc.tile_pool(name="qps", bufs=2, space="PSUM") as qps, \
             tc.tile_pool(name="qkps", bufs=2, space="PSUM") as qkps:
            for m in range(H):
                wqt = wqp.tile([128, 12, 128], F32, tag="wq")
                nc.sync.dma_start(out=wqt, in_=wq[m])
                ps_q = qps.tile([128, 256], F32, tag="qps")
                for kc in range(12):
                    nc.tensor.matmul(ps_q, wqt[:, kc, :], qrT[:, kc, :],
                                     start=(kc == 0), stop=(kc == 11))
                q_sb = qro_p.tile([128, 256], F32, tag="q_sb")
                nc.scalar.copy(q_sb, ps_q)
                q_sw = qro_p.tile([128, 256], F32, tag="q_sw")
                nc.sync.dma_start(out=q_sw[0:64, :], in_=q_sb[64:128, :])
                nc.sync.dma_start(out=q_sw[64:128, :], in_=q_sb[0:64, :])
                qro = qro_p.tile([128, 256], F32, tag="qro")
                tmp = qro_p.tile([128, 256], F32, tag="qtmp")
                nc.vector.tensor_mul(qro, q_sb, cc)
                nc.gpsimd.tensor_mul(tmp, q_sw, ss)
                nc.vector.tensor_add(qro, qro, tmp)
                for tt in range(2):
                    ps_qk = qkps.tile([128, C], F32, tag="qkps")
                    nc.tensor.matmul(ps_qk, qro[:, tt * 128:(tt + 1) * 128],
                                     kT, start=True, stop=True)
                    if m == 0:
                        nc.vector.tensor_scalar(
                            acc[tt], ps_qk, 0.0, wts_sb[tt][:, m:m + 1],
                            op0=mybir.AluOpType.max, op1=mybir.AluOpType.mult)
                    else:
                        rl = work.tile([128, C], F32, tag=f"rl{tt}")
                        nc.vector.tensor_scalar(
                            rl, ps_qk, 0.0, wts_sb[tt][:, m:m + 1],
                            op0=mybir.AluOpType.max, op1=mybir.AluOpType.mult)
                        nc.gpsimd.tensor_add(acc[tt], acc[tt], rl)

        # ---- causal mask + top-k ----
        for tt in range(2):
            W = WIDTHS[tt]
            cmp = work.tile([128, C], F32, tag="cmp")
            nc.vector.tensor_scalar(cmp[:, :W], c4f[:, :W], pos_sb[tt], None,
                                    op0=mybir.AluOpType.add)
            mbit = work.tile([128, C], U32, tag="mbit")
            nc.vector.tensor_scalar(mbit[:, :W], cmp[:, :W], 0.0, None,
                                    op0=mybir.AluOpType.is_lt)
            nc.vector.copy_predicated(acc[tt][:, :W], mbit[:, :W],
                                      negs[:, :W])

            idx = tk.tile([128, TOPK], U32, tag="idx")
            vals = acc[tt]
            for it in range(32):
                mx = tk.tile([128, 8], F32, tag="mx")
                mi = tk.tile([128, 8], U32, tag="mi")
                nc.vector.max(out=mx, in_=vals[:, :W])
                nc.vector.max_index(out=mi, in_max=mx, in_values=vals[:, :W])
                nc.vector.match_replace(out=vals[:, :W], in_to_replace=mx,
                                        in_values=vals[:, :W], imm_value=NEG)
                nc.vector.tensor_copy(idx[:, it * 8:(it + 1) * 8], mi)

            idx32 = tk.tile([128, TOPK], I32, tag="idx32")
            nc.vector.tensor_copy(idx32, idx)
            rmp = work.tile([128, TOPK], F32, tag="rmp")
            nc.vector.tensor_scalar(rmp, c4f[:, :TOPK], pos_sb[tt], None,
                                    op0=mybir.AluOpType.add)
            rbit = work.tile([128, TOPK], U32, tag="rbit")
            nc.vector.tensor_scalar(rbit, rmp, 0.0, None,
                                    op0=mybir.AluOpType.is_lt)
            nc.vector.copy_predicated(idx32, rbit, neg1)
            nc.sync.dma_start(out=out_idx[tt * 128:(tt + 1) * 128, :],
                              in_=idx32)

    nc.finalize()
    return nc


def _get(name):
    if name not in _cache:
        _cache[name] = _build_l1() if name == "l1" else _build_l2()
    return _cache[name]


def kernel(hidden_states, qr, positions, W_fused, Wq, Wproj, ape, rms_weight,
           cos_sin_cache, _timing=None):
    hidden_states = np.asarray(hidden_states, np.float32)
    qr = np.asarray(qr, np.float32)
    positions = np.asarray(positions, np.int32)
    W_fused = np.asarray(W_fused, np.float32)
    Wq = np.asarray(Wq, np.float32)
    Wproj = np.asarray(Wproj, np.float32)
    ape = np.asarray(ape, np.float32)
    rms_weight = np.asarray(rms_weight, np.float32)
    cos_sin_cache = np.asarray(cos_sin_cache, np.float32)

    wcomb = np.ascontiguousarray(
        np.concatenate([W_fused.T, Wproj.T], axis=1))          # [7168, 576]
    wq_pre = np.ascontiguousarray(
        Wq.reshape(H, 128, 12, 128).transpose(0, 3, 2, 1))     # [m, kk, kc, mm]

    cores = list(range(NC))
    trace = _timing is not None

    in1 = []
    for i in cores:
        rows = []
        for j in PAIRS[i]:
            lo = 128 * j - 4
            if lo < 0:
                blk = np.zeros((132, HID), np.float32)
                blk[4:] = hidden_states[0:128 * j + 128]
            else:
                blk = hidden_states[lo:128 * j + 128]
            rows.append(blk)
        hidden_halo = np.ascontiguousarray(np.concatenate(rows, axis=0))
        cs_rows = np.concatenate(
            [cos_sin_cache[32 * j:32 * j + 32] for j in PAIRS[i]], axis=0)
        hfl = np.array([NEG if j == 0 else 0.0 for j in PAIRS[i]], np.float32)
        in1.append({
            "hidden": hidden_halo, "wcomb": wcomb, "ape": ape,
            "rmsw": rms_weight, "cs_k": np.ascontiguousarray(cs_rows),
            "haloflag": hfl,
        })
    r1 = run_bass_kernel_spmd(_get("l1"), in1, cores, trace=trace,
                              trace_cores=cores if trace else None)

    k_full = np.zeros((C, D), np.float32)
    wts = {}
    for i in cores:
        kl = r1.results[i]["k_loc"]
        for s, j in enumerate(PAIRS[i]):
            k_full[32 * j:32 * j + 32] = kl[32 * s:32 * s + 32]
        wts[i] = r1.results[i]["wts_own"]

    in2 = []
    for i in cores:
        sel = np.concatenate(
            [np.arange(128 * j, 128 * j + 128) for j in PAIRS[i]])
        in2.append({
            "qr_sh": np.ascontiguousarray(qr[sel]),
            "wq": wq_pre,
            "cs_own": np.ascontiguousarray(cos_sin_cache[positions[sel]]),
            "k_full": k_full,
            "wts_own": wts[i],
            "posm3": (positions[sel] - 3).astype(np.float32),
        })
    r2 = run_bass_kernel_spmd(_get("l2"), in2, cores, trace=trace,
                              trace_cores=cores if trace else None)

    out = np.empty((T, TOPK), np.int32)
    for i in cores:
        oi = r2.results[i]["out_idx"]
        for s, j in enumerate(PAIRS[i]):
            out[128 * j:128 * j + 128] = oi[128 * s:128 * s + 128]

    if _timing is not None:
        _timing["l1"] = r1
        _timing["l2"] = r2
    return out


# revision 10
# speedup vs baseline: 1.1773x; 1.1773x over previous
"""DeepseekV4 indexer kernel for 8 trn2 NeuronCores (Bass/Tile).

Strategy (token-sharded, two bass launches):
  - Tokens are split into 16 tiles of 128; core i owns tiles (i, 15-i) so the
    causally-pruned top-k work is balanced across cores.
  - Launch 1 (per core): fused W_fused/Wproj GEMM over the core's 264-token
    halo'd shard -> compressor softmax -> RMSNorm -> RoPE -> compressed K for
    its 64 compressed positions, plus per-token head weights.  Outputs are
    tiny ([64,128] K + [256,64] wts per core).
  - Host: concatenates the per-core K shards into the full [512,128] K
    (the "all-gather"; collectives don't load on this runtime).
  - Launch 2 (per core): q = qr @ Wq.T (fp32), RoPE, qk against full K,
    relu * wts accumulation over 64 heads, causal mask, and iterated
    top-8 (max / max_index / match_replace) producing the top-256 indices
    in descending-score order.  All GEMMs run in exact fp32 so the ordering
    matches the fp32 reference up to fp32 rounding noise.

kernel(**inputs) takes the FULL unsharded inputs and returns [2048,256] int32.
"""
import sys
sys.path.insert(0, '/opt/trn_rl_repo')

from contextlib import ExitStack

import numpy as np

import concourse.bass as bass
import concourse.bacc as bacc
import concourse.tile as tile
from concourse import mybir
from concourse.bass_utils import run_bass_kernel_spmd
from concourse.masks import make_identity

T, HID, QR_DIM, H, D, TOPK, R = 2048, 7168, 1536, 64, 128, 256, 4
C = T // R
NC = 8
EPS = 1e-6
F32 = mybir.dt.float32
I32 = mybir.dt.int32
U32 = mybir.dt.uint32
WTS_SCALE = float(H ** -0.5) * float(D ** -0.5)  # folds q's D**-0.5 into wts
NEG = -1e30

PAIRS = [(i, 15 - i) for i in range(NC)]  # token tiles owned by core i

_cache = {}


# --------------------------------------------------------------------------
# launch 1: compressor -> per-core compressed K (64 rows) + head weights
# --------------------------------------------------------------------------
def _build_l1():
    nc = bacc.Bacc()
    hidden = nc.declare_dram_parameter("hidden", [264, HID], F32, isOutput=False)
    wcomb = nc.declare_dram_parameter("wcomb", [HID, 576], F32, isOutput=False)
    ape = nc.declare_dram_parameter("ape", [8, D], F32, isOutput=False)
    rmsw = nc.declare_dram_parameter("rmsw", [D], F32, isOutput=False)
    cs_k = nc.declare_dram_parameter("cs_k", [64, D], F32, isOutput=False)
    haloflag = nc.declare_dram_parameter("haloflag", [2], F32, isOutput=False)
    k_loc = nc.declare_dram_parameter("k_loc", [64, D], F32, isOutput=True)
    wts_own = nc.declare_dram_parameter("wts_own", [256, H], F32, isOutput=True)

    with tile.TileContext(nc) as tc, ExitStack() as ctx:
        const = ctx.enter_context(tc.tile_pool(name="const", bufs=1))
        big = ctx.enter_context(tc.tile_pool(name="big", bufs=1))
        work = ctx.enter_context(tc.tile_pool(name="work", bufs=2))

        ident = const.tile([128, 128], F32)
        make_identity(nc, ident)

        def tp(ps_out, in_sb):
            p = in_sb.shape[0]
            nc.tensor.transpose(ps_out, in_sb, ident[:p, :p])

        # ---- hiddenT [128, 56, 264] via PE transposes ----
        hidT = big.tile([128, 56, 264], F32)
        with tc.tile_pool(name="stg", bufs=2) as stg, \
             tc.tile_pool(name="tpsA", bufs=2, space="PSUM") as tpsA:
            for (t0, rows) in [(0, 128), (128, 128), (256, 8)]:
                stage = stg.tile([128, HID], F32, tag="stage")
                nc.sync.dma_start(out=stage[:rows, :], in_=hidden[t0:t0 + rows, :])
                for kg in range(14):
                    ps = tpsA.tile([128, 512], F32, tag="tp")
                    for u in range(4):
                        kc = kg * 4 + u
                        tp(ps[:, u * 128:u * 128 + rows],
                           stage[:rows, kc * 128:(kc + 1) * 128])
                    sv = ps.rearrange("p (u x) -> p u x", x=128)[:, :, :rows]
                    nc.scalar.copy(hidT[:, kg * 4:kg * 4 + 4, t0:t0 + rows], sv)

        # ---- fused GEMM: kv_scoreT [4x128, 264] + wtsT [64, 264] ----
        kvt = []
        wts_sb = work.tile([64, 264], F32, tag="wts_sb")
        with tc.tile_pool(name="wstg", bufs=3) as wstg, \
             tc.tile_pool(name="gps", bufs=1, space="PSUM") as gps:
            kvps = [gps.tile([128, 264], F32, tag=f"kvps{m}", name=f"kvps{m}") for m in range(4)]
            wtsps = gps.tile([64, 264], F32, tag="wtsps")
            for kc in range(56):
                wt = wstg.tile([128, 576], F32, tag="wcomb")
                nc.sync.dma_start(out=wt, in_=wcomb[kc * 128:(kc + 1) * 128, :])
                for m in range(4):
                    nc.tensor.matmul(kvps[m], wt[:, m * 128:(m + 1) * 128],
                                     hidT[:, kc, :], start=(kc == 0),
                                     stop=(kc == 55))
                nc.tensor.matmul(wtsps, wt[:, 512:576], hidT[:, kc, :],
                                 start=(kc == 0), stop=(kc == 55))
            for m in range(4):
                t = work.tile([128, 264], F32, tag=f"kvt{m}")
                nc.scalar.copy(t, kvps[m])
                kvt.append(t)
            nc.scalar.mul(wts_sb, wtsps, WTS_SCALE)
        kv_old, kv_new, sc_old, sc_new = kvt

        with tc.tile_pool(name="tpsB", bufs=2, space="PSUM") as tpsB:
            # wts -> [t, h] and out
            for s in range(2):
                ps = tpsB.tile([128, 64], F32, tag="wtp")
                tp(ps, wts_sb[:, 4 + 132 * s:132 + 132 * s])
                ob = work.tile([128, 64], F32, tag="wob")
                nc.scalar.copy(ob, ps)
                nc.sync.dma_start(out=wts_own[128 * s:128 * (s + 1), :], in_=ob)

            # ape transposed + replicated [128, 32, 8]
            ape_st = work.tile([8, D], F32, tag="ape_st")
            nc.sync.dma_start(out=ape_st, in_=ape[:])
            aps = tpsB.tile([128, 8], F32, tag="apetp")
            tp(aps, ape_st)
            apeT = const.tile([128, 8], F32)
            nc.scalar.copy(apeT, aps)
            ape_rep = const.tile([128, 32, 8], F32)
            for g in range(32):
                nc.vector.tensor_copy(ape_rep[:, g, :], apeT)

            # rms weight replicated [32, 128]
            rms_rep = const.tile([32, D], F32)
            nc.sync.dma_start(out=rms_rep, in_=bass.AP(
                tensor=rmsw, offset=0, ap=[[0, 32], [1, D]]))

            cs_st = []
            for s in range(2):
                cst = const.tile([32, D], F32, tag=f"cs{s}", name=f"cs{s}")
                nc.sync.dma_start(out=cst, in_=cs_k[32 * s:32 * s + 32, :])
                cs_st.append(cst)

            hf = []
            for s in range(2):
                h = const.tile([128, 1], F32, tag=f"hf{s}")
                nc.sync.dma_start(out=h, in_=bass.AP(
                    tensor=haloflag, offset=s, ap=[[0, 128], [1, 1]]))
                hf.append(h)

            for s in range(2):
                o = 132 * s
                gates = work.tile([128, 32, 8], F32, tag="gates")
                so_v = sc_old[:, o:o + 128].rearrange("p (g x) -> p g x", x=4)
                sn_v = sc_new[:, o + 4:o + 132].rearrange("p (g x) -> p g x", x=4)
                ko_v = kv_old[:, o:o + 128].rearrange("p (g x) -> p g x", x=4)
                kn_v = kv_new[:, o + 4:o + 132].rearrange("p (g x) -> p g x", x=4)
                nc.vector.tensor_add(gates[:, :, 0:4], so_v, ape_rep[:, :, 0:4])
                nc.vector.tensor_add(gates[:, :, 4:8], sn_v, ape_rep[:, :, 4:8])
                # first group's old slots += -1e30 when strip starts at t=0
                nc.vector.tensor_scalar(gates[:, 0, 0:4], gates[:, 0, 0:4],
                                        hf[s], None, op0=mybir.AluOpType.add)
                gmax = work.tile([128, 32], F32, tag="gmax")
                nc.vector.reduce_max(gmax, gates, axis=mybir.AxisListType.X)
                nc.vector.tensor_sub(gates, gates,
                                     gmax.to_broadcast([128, 32, 8]))
                ex = work.tile([128, 32, 8], F32, tag="ex")
                nc.scalar.activation(ex, gates, mybir.ActivationFunctionType.Exp)
                den = work.tile([128, 32], F32, tag="den")
                nc.vector.reduce_sum(den, ex, axis=mybir.AxisListType.X)
                rec = work.tile([128, 32], F32, tag="rec")
                nc.vector.reciprocal(rec, den)
                w8 = work.tile([128, 32, 8], F32, tag="w8")
                nc.vector.tensor_mul(w8, ex, rec.to_broadcast([128, 32, 8]))
                prod = work.tile([128, 32, 8], F32, tag="prod")
                nc.vector.tensor_mul(prod[:, :, 0:4], w8[:, :, 0:4], ko_v)
                nc.vector.tensor_mul(prod[:, :, 4:8], w8[:, :, 4:8], kn_v)
                comp = work.tile([128, 32], F32, tag="comp")
                nc.vector.reduce_sum(comp, prod, axis=mybir.AxisListType.X)

                cps = tpsB.tile([32, 128], F32, tag="ctp")
                tp(cps, comp)
                compT = work.tile([32, D], F32, tag="compT")
                nc.scalar.copy(compT, cps)

                # RMSNorm over d
                sq = work.tile([32, D], F32, tag="sq")
                nc.vector.tensor_mul(sq, compT, compT)
                ssum = work.tile([32, 1], F32, tag="ssum")
                nc.vector.reduce_sum(ssum, sq, axis=mybir.AxisListType.X)
                nc.vector.tensor_scalar(ssum, ssum, 1.0 / D, EPS,
                                        op0=mybir.AluOpType.mult,
                                        op1=mybir.AluOpType.add)
                rt = work.tile([32, 1], F32, tag="rt")
                nc.scalar.sqrt(rt, ssum)
                rs = work.tile([32, 1], F32, tag="rs")
                nc.vector.reciprocal(rs, rt)
                nc.vector.tensor_scalar(compT, compT, rs, None,
                                        op0=mybir.AluOpType.mult)
                nc.vector.tensor_mul(compT, compT, rms_rep)

                # RoPE at compressed positions (all tiles at base partition 0)
                co = cs_st[s][:, 0:64]
                si = cs_st[s][:, 64:128]
                x1 = compT[:, 0:64]
                x2 = compT[:, 64:128]
                tmp = work.tile([32, D], F32, tag="ktmp")
                kx = work.tile([32, D], F32, tag="kx")
                nc.vector.tensor_mul(kx[:, 0:64], x1, co)
                nc.vector.tensor_mul(tmp[:, 0:64], x2, si)
                nc.vector.tensor_sub(kx[:, 0:64], kx[:, 0:64], tmp[:, 0:64])
                nc.vector.tensor_mul(kx[:, 64:128], x2, co)
                nc.vector.tensor_mul(tmp[:, 64:128], x1, si)
                nc.vector.tensor_add(kx[:, 64:128], kx[:, 64:128],
                                     tmp[:, 64:128])
                nc.sync.dma_start(out=k_loc[32 * s:32 * s + 32, :], in_=kx)

    nc.finalize()
    return nc


# --------------------------------------------------------------------------
# launch 2: q GEMM + RoPE + qk + score assembly + mask + top-k
# --------------------------------------------------------------------------
def _build_l2():
    nc = bacc.Bacc()
    qr_sh = nc.declare_dram_parameter("qr_sh", [256, QR_DIM], F32, isOutput=False)
    wq = nc.declare_dram_parameter("wq", [H, 128, 12, 128], F32, isOutput=False)
    cs_own = nc.declare_dram_parameter("cs_own", [256, D], F32, isOutput=False)
    k_full = nc.declare_dram_parameter("k_full", [C, D], F32, isOutput=False)
    wts_own = nc.declare_dram_parameter("wts_own", [256, H], F32, isOutput=False)
    posm3 = nc.declare_dram_parameter("posm3", [256], F32, isOutput=False)
    out_idx = nc.declare_dram_parameter("out_idx", [256, TOPK], I32, isOutput=True)

    WIDTHS = (256, 512)  # candidate widths for (low tile j<=7, high tile)

    with tile.TileContext(nc) as tc, ExitStack() as ctx:
        const = ctx.enter_context(tc.tile_pool(name="const", bufs=1))
        work = ctx.enter_context(tc.tile_pool(name="work", bufs=2))
        tk = ctx.enter_context(tc.tile_pool(name="tk", bufs=2))

        ident = const.tile([128, 128], F32)
        make_identity(nc, ident)

        def tp(ps_out, in_sb):
            p = in_sb.shape[0]
            nc.tensor.transpose(ps_out, in_sb, ident[:p, :p])

        qrT = const.tile([128, 12, 256], F32)
        csT = const.tile([128, 256], F32)
        kT = const.tile([128, C], F32)
        with tc.tile_pool(name="stg", bufs=2) as stg, \
             tc.tile_pool(name="tps", bufs=2, space="PSUM") as tps:
            for tt in range(2):
                stage = stg.tile([128, QR_DIM], F32, tag="qstage")
                nc.sync.dma_start(out=stage,
                                  in_=qr_sh[tt * 128:(tt + 1) * 128, :])
                for kg in range(3):
                    ps = tps.tile([128, 512], F32, tag="tp")
                    for u in range(4):
                        kc = kg * 4 + u
                        tp(ps[:, u * 128:(u + 1) * 128],
                           stage[:, kc * 128:(kc + 1) * 128])
                    nc.scalar.copy(
                        qrT[:, kg * 4:kg * 4 + 4, tt * 128:(tt + 1) * 128],
                        ps.rearrange("p (u x) -> p u x", x=128))
            for tt in range(2):
                stage = stg.tile([128, D], F32, tag="cstage")
                nc.sync.dma_start(out=stage,
                                  in_=cs_own[tt * 128:(tt + 1) * 128, :])
                ps = tps.tile([128, 512], F32, tag="tp")
                tp(ps[:, :128], stage)
                nc.scalar.copy(csT[:, tt * 128:(tt + 1) * 128], ps[:, :128])
            kstage = const.tile([128, 4, D], F32)
            nc.sync.dma_start(out=kstage,
                              in_=k_full[:].rearrange("(a p) d -> p a d", p=128))
            for a in range(4):
                ps = tps.tile([128, 512], F32, tag="tp")
                tp(ps[:, :128], kstage[:, a, :])
                nc.scalar.copy(kT[:, a * 128:(a + 1) * 128], ps[:, :128])

        # cc = [cos;cos], ss = [-sin;sin] (partition moves via DMA only)
        cc = const.tile([128, 256], F32)
        ss = const.tile([128, 256], F32)
        nc.sync.dma_start(out=cc[0:64, :], in_=csT[0:64, :])
        nc.sync.dma_start(out=cc[64:128, :], in_=csT[0:64, :])
        nc.sync.dma_start(out=ss[0:64, :], in_=csT[64:128, :])
        nc.sync.dma_start(out=ss[64:128, :], in_=csT[64:128, :])
        nc.vector.tensor_scalar(ss[0:64, :], ss[0:64, :], -1.0, None,
                                op0=mybir.AluOpType.mult)

        wts_sb, pos_sb = [], []
        for tt in range(2):
            w = const.tile([128, H], F32, tag=f"wts{tt}")
            nc.sync.dma_start(out=w, in_=wts_own[tt * 128:(tt + 1) * 128, :])
            wts_sb.append(w)
            p = const.tile([128, 1], F32, tag=f"pos{tt}")
            nc.sync.dma_start(out=p, in_=posm3[tt * 128:(tt + 1) * 128])
            pos_sb.append(p)

        c4p = const.tile([128, C], F32)
        nc.gpsimd.iota(c4p, pattern=[[4, C]], base=0, channel_multiplier=0,
                       allow_small_or_imprecise_dtypes=True)
        c4f = const.tile([128, C], F32)
        nc.vector.tensor_scalar(c4f, c4p, -1.0, None, op0=mybir.AluOpType.mult)
        negs = const.tile([128, C], F32)
        nc.vector.memset(negs, NEG)
        neg1 = const.tile([128, TOPK], I32)
        nc.vector.memset(neg1, -1)

        acc = [const.tile([128, C], F32, tag=f"acc{tt}", name=f"acc{tt}") for tt in range(2)]

        with tc.tile_pool(name="wqp", bufs=3) as wqp, \
             tc.tile_pool(name="qro_p", bufs=2) as qro_p, \
             tc.tile_pool(name="qps", bufs=2, space="PSUM") as qps, \
             tc.tile_pool(name="qkps", bufs=2, space="PSUM") as qkps:
            for m in range(H):
                wqt = wqp.tile([128, 12, 128], F32, tag="wq")
                nc.sync.dma_start(out=wqt, in_=wq[m])
                ps_q = qps.tile([128, 256], F32, tag="qps")
                for kc in range(12):
                    nc.tensor.matmul(ps_q, wqt[:, kc, :], qrT[:, kc, :],
                                     start=(kc == 0), stop=(kc == 11))
                q_sb = qro_p.tile([128, 256], F32, tag="q_sb")
                nc.scalar.copy(q_sb, ps_q)
                q_sw = qro_p.tile([128, 256], F32, tag="q_sw")
                nc.sync.dma_start(out=q_sw[0:64, :], in_=q_sb[64:128, :])
                nc.sync.dma_start(out=q_sw[64:128, :], in_=q_sb[0:64, :])
                qro = qro_p.tile([128, 256], F32, tag="qro")
                tmp = qro_p.tile([128, 256], F32, tag="qtmp")
                nc.vector.tensor_mul(qro, q_sb, cc)
                nc.vector.tensor_mul(tmp, q_sw, ss)
                nc.vector.tensor_add(qro, qro, tmp)
                for tt in range(2):
                    Wtt = WIDTHS[tt]
                    ps_qk = qkps.tile([128, Wtt], F32, tag=f"qkps{tt}",
                                      name=f"ps_qk{tt}")
                    nc.tensor.matmul(ps_qk, qro[:, tt * 128:(tt + 1) * 128],
                                     kT[:, :Wtt], start=True, stop=True)
                    if m == 0:
                        nc.vector.tensor_scalar(
                            acc[tt][:, :Wtt], ps_qk, 0.0,
                            wts_sb[tt][:, m:m + 1],
                            op0=mybir.AluOpType.max, op1=mybir.AluOpType.mult)
                    else:
                        rl = work.tile([128, Wtt], F32, tag=f"rl{tt}",
                                       name=f"rl{tt}")
                        nc.vector.tensor_scalar(
                            rl, ps_qk, 0.0, wts_sb[tt][:, m:m + 1],
                            op0=mybir.AluOpType.max, op1=mybir.AluOpType.mult)
                        nc.vector.tensor_add(acc[tt][:, :Wtt],
                                             acc[tt][:, :Wtt], rl)

        # ---- causal mask + top-k ----
        for tt in range(2):
            W = WIDTHS[tt]
            cmp = work.tile([128, C], F32, tag="cmp")
            nc.vector.tensor_scalar(cmp[:, :W], c4f[:, :W], pos_sb[tt], None,
                                    op0=mybir.AluOpType.add)
            mbit = work.tile([128, C], U32, tag="mbit")
            nc.vector.tensor_scalar(mbit[:, :W], cmp[:, :W], 0.0, None,
                                    op0=mybir.AluOpType.is_lt)
            nc.vector.copy_predicated(acc[tt][:, :W], mbit[:, :W],
                                      negs[:, :W])

            idx = tk.tile([128, TOPK], U32, tag="idx")
            vals = acc[tt]
            for it in range(32):
                mx = tk.tile([128, 8], F32, tag="mx")
                nc.vector.max(out=mx, in_=vals[:, :W])
                nc.vector.max_index(out=idx[:, it * 8:(it + 1) * 8],
                                    in_max=mx, in_values=vals[:, :W])
                nc.vector.match_replace(out=vals[:, :W], in_to_replace=mx,
                                        in_values=vals[:, :W], imm_value=NEG)

            idx32 = tk.tile([128, TOPK], I32, tag="idx32")
            nc.vector.tensor_copy(idx32, idx)
            rmp = work.tile([128, TOPK], F32, tag="rmp")
            nc.vector.tensor_scalar(rmp, c4f[:, :TOPK], pos_sb[tt], None,
                                    op0=mybir.AluOpType.add)
            rbit = work.tile([128, TOPK], U32, tag="rbit")
            nc.vector.tensor_scalar(rbit, rmp, 0.0, None,
                                    op0=mybir.AluOpType.is_lt)
            nc.vector.copy_predicated(idx32, rbit, neg1)
            nc.sync.dma_start(out=out_idx[tt * 128:(tt + 1) * 128, :],
                              in_=idx32)

    nc.finalize()
    return nc


def _get(name):
    if name not in _cache:
        _cache[name] = _build_l1() if name == "l1" else _build_l2()
    return _cache[name]


def kernel(hidden_states, qr, positions, W_fused, Wq, Wproj, ape, rms_weight,
           cos_sin_cache, _timing=None):
    hidden_states = np.asarray(hidden_states, np.float32)
    qr = np.asarray(qr, np.float32)
    positions = np.asarray(positions, np.int32)
    W_fused = np.asarray(W_fused, np.float32)
    Wq = np.asarray(Wq, np.float32)
    Wproj = np.asarray(Wproj, np.float32)
    ape = np.asarray(ape, np.float32)
    rms_weight = np.asarray(rms_weight, np.float32)
    cos_sin_cache = np.asarray(cos_sin_cache, np.float32)

    wcomb = np.ascontiguousarray(
        np.concatenate([W_fused.T, Wproj.T], axis=1))          # [7168, 576]
    wq_pre = np.ascontiguousarray(
        Wq.reshape(H, 128, 12, 128).transpose(0, 3, 2, 1))     # [m, kk, kc, mm]

    cores = list(range(NC))
    trace = _timing is not None

    in1 = []
    for i in cores:
        rows = []
        for j in PAIRS[i]:
            lo = 128 * j - 4
            if lo < 0:
                blk = np.zeros((132, HID), np.float32)
                blk[4:] = hidden_states[0:128 * j + 128]
            else:
                blk = hidden_states[lo:128 * j + 128]
            rows.append(blk)
        hidden_halo = np.ascontiguousarray(np.concatenate(rows, axis=0))
        cs_rows = np.concatenate(
            [cos_sin_cache[32 * j:32 * j + 32] for j in PAIRS[i]], axis=0)
        hfl = np.array([NEG if j == 0 else 0.0 for j in PAIRS[i]], np.float32)
        in1.append({
            "hidden": hidden_halo, "wcomb": wcomb, "ape": ape,
            "rmsw": rms_weight, "cs_k": np.ascontiguousarray(cs_rows),
            "haloflag": hfl,
        })
    r1 = run_bass_kernel_spmd(_get("l1"), in1, cores, trace=trace,
                              trace_cores=cores if trace else None)

    k_full = np.zeros((C, D), np.float32)
    wts = {}
    for i in cores:
        kl = r1.results[i]["k_loc"]
        for s, j in enumerate(PAIRS[i]):
            k_full[32 * j:32 * j + 32] = kl[32 * s:32 * s + 32]
        wts[i] = r1.results[i]["wts_own"]

    in2 = []
    for i in cores:
        sel = np.concatenate(
            [np.arange(128 * j, 128 * j + 128) for j in PAIRS[i]])
        in2.append({
            "qr_sh": np.ascontiguousarray(qr[sel]),
            "wq": wq_pre,
            "cs_own": np.ascontiguousarray(cos_sin_cache[positions[sel]]),
            "k_full": k_full,
            "wts_own": wts[i],
            "posm3": (positions[sel] - 3).astype(np.float32),
        })
    r2 = run_bass_kernel_spmd(_get("l2"), in2, cores, trace=trace,
                              trace_cores=cores if trace else None)

    out = np.empty((T, TOPK), np.int32)
    for i in cores:
        oi = r2.results[i]["out_idx"]
        for s, j in enumerate(PAIRS[i]):
            out[128 * j:128 * j + 128] = oi[128 * s:128 * s + 128]

    if _timing is not None:
        _timing["l1"] = r1
        _timing["l2"] = r2
    return out


# revision 14
# speedup vs baseline: 1.2462x; 1.0585x over previous
"""DeepseekV4 indexer kernel for 8 trn2 NeuronCores (Bass/Tile).

Strategy (token-sharded, two bass launches):
  - Tokens are split into 16 tiles of 128; core i owns tiles (i, 15-i) so the
    causally-pruned top-k work is balanced across cores.
  - Launch 1 (per core): fused W_fused/Wproj GEMM over the core's 264-token
    halo'd shard -> compressor softmax -> RMSNorm -> RoPE -> compressed K for
    its 64 compressed positions, plus per-token head weights.  Outputs are
    tiny ([64,128] K + [256,64] wts per core).
  - Host: concatenates the per-core K shards into the full [512,128] K
    (the "all-gather"; collectives don't load on this runtime).
  - Launch 2 (per core): q = qr @ Wq.T (fp32), RoPE, qk against full K,
    relu * wts accumulation over 64 heads, causal mask, and iterated
    top-8 (max / max_index / match_replace) producing the top-256 indices
    in descending-score order.  All GEMMs run in exact fp32 so the ordering
    matches the fp32 reference up to fp32 rounding noise.

kernel(**inputs) takes the FULL unsharded inputs and returns [2048,256] int32.
"""
import sys
sys.path.insert(0, '/opt/trn_rl_repo')

from contextlib import ExitStack

import numpy as np

import concourse.bass as bass
import concourse.bacc as bacc
import concourse.tile as tile
from concourse import mybir
from concourse.bass_utils import run_bass_kernel_spmd
from concourse.masks import make_identity

T, HID, QR_DIM, H, D, TOPK, R = 2048, 7168, 1536, 64, 128, 256, 4
C = T // R
NC = 8
EPS = 1e-6
F32 = mybir.dt.float32
I32 = mybir.dt.int32
U32 = mybir.dt.uint32
WTS_SCALE = float(H ** -0.5) * float(D ** -0.5)  # folds q's D**-0.5 into wts
NEG = -1e30

PAIRS = [(i, 15 - i) for i in range(NC)]  # token tiles owned by core i

_cache = {}


# --------------------------------------------------------------------------
# launch 1: compressor -> per-core compressed K (64 rows) + head weights
# --------------------------------------------------------------------------
def _build_l1():
    nc = bacc.Bacc()
    hidden = nc.declare_dram_parameter("hidden", [264, HID], F32, isOutput=False)
    wcomb = nc.declare_dram_parameter("wcomb", [HID, 576], F32, isOutput=False)
    ape = nc.declare_dram_parameter("ape", [8, D], F32, isOutput=False)
    rmsw = nc.declare_dram_parameter("rmsw", [D], F32, isOutput=False)
    cs_k = nc.declare_dram_parameter("cs_k", [64, D], F32, isOutput=False)
    haloflag = nc.declare_dram_parameter("haloflag", [2], F32, isOutput=False)
    k_loc = nc.declare_dram_parameter("k_loc", [64, D], F32, isOutput=True)
    wts_own = nc.declare_dram_parameter("wts_own", [256, H], F32, isOutput=True)

    with tile.TileContext(nc) as tc, ExitStack() as ctx:
        const = ctx.enter_context(tc.tile_pool(name="const", bufs=1))
        big = ctx.enter_context(tc.tile_pool(name="big", bufs=1))
        work = ctx.enter_context(tc.tile_pool(name="work", bufs=2))

        ident = const.tile([128, 128], F32)
        make_identity(nc, ident)

        def tp(ps_out, in_sb):
            p = in_sb.shape[0]
            nc.tensor.transpose(ps_out, in_sb, ident[:p, :p])

        # ---- hiddenT [128, 56, 264] via PE transposes ----
        hidT = big.tile([128, 56, 264], F32)
        with tc.tile_pool(name="stg", bufs=2) as stg, \
             tc.tile_pool(name="tpsA", bufs=2, space="PSUM") as tpsA:
            for (t0, rows) in [(0, 128), (128, 128), (256, 8)]:
                stage = stg.tile([128, HID], F32, tag="stage")
                nc.sync.dma_start(out=stage[:rows, :], in_=hidden[t0:t0 + rows, :])
                for kg in range(14):
                    ps = tpsA.tile([128, 512], F32, tag="tp")
                    for u in range(4):
                        kc = kg * 4 + u
                        tp(ps[:, u * 128:u * 128 + rows],
                           stage[:rows, kc * 128:(kc + 1) * 128])
                    sv = ps.rearrange("p (u x) -> p u x", x=128)[:, :, :rows]
                    nc.scalar.copy(hidT[:, kg * 4:kg * 4 + 4, t0:t0 + rows], sv)

        # ---- fused GEMM: kv_scoreT [4x128, 264] + wtsT [64, 264] ----
        kvt = []
        wts_sb = work.tile([64, 264], F32, tag="wts_sb")
        with tc.tile_pool(name="wstg", bufs=3) as wstg, \
             tc.tile_pool(name="gps", bufs=1, space="PSUM") as gps:
            kvps = [gps.tile([128, 264], F32, tag=f"kvps{m}", name=f"kvps{m}") for m in range(4)]
            wtsps = gps.tile([64, 264], F32, tag="wtsps")
            for kc in range(56):
                wt = wstg.tile([128, 576], F32, tag="wcomb")
                nc.sync.dma_start(out=wt, in_=wcomb[kc * 128:(kc + 1) * 128, :])
                for m in range(4):
                    nc.tensor.matmul(kvps[m], wt[:, m * 128:(m + 1) * 128],
                                     hidT[:, kc, :], start=(kc == 0),
                                     stop=(kc == 55))
                nc.tensor.matmul(wtsps, wt[:, 512:576], hidT[:, kc, :],
                                 start=(kc == 0), stop=(kc == 55))
            for m in range(4):
                t = work.tile([128, 264], F32, tag=f"kvt{m}")
                nc.scalar.copy(t, kvps[m])
                kvt.append(t)
            nc.scalar.mul(wts_sb, wtsps, WTS_SCALE)
        kv_old, kv_new, sc_old, sc_new = kvt

        with tc.tile_pool(name="tpsB", bufs=2, space="PSUM") as tpsB:
            # wts -> [t, h] and out
            for s in range(2):
                ps = tpsB.tile([128, 64], F32, tag="wtp")
                tp(ps, wts_sb[:, 4 + 132 * s:132 + 132 * s])
                ob = work.tile([128, 64], F32, tag="wob")
                nc.scalar.copy(ob, ps)
                nc.sync.dma_start(out=wts_own[128 * s:128 * (s + 1), :], in_=ob)

            # ape transposed + replicated [128, 32, 8]
            ape_st = work.tile([8, D], F32, tag="ape_st")
            nc.sync.dma_start(out=ape_st, in_=ape[:])
            aps = tpsB.tile([128, 8], F32, tag="apetp")
            tp(aps, ape_st)
            apeT = const.tile([128, 8], F32)
            nc.scalar.copy(apeT, aps)
            ape_rep = const.tile([128, 32, 8], F32)
            for g in range(32):
                nc.vector.tensor_copy(ape_rep[:, g, :], apeT)

            # rms weight replicated [32, 128]
            rms_rep = const.tile([32, D], F32)
            nc.sync.dma_start(out=rms_rep, in_=bass.AP(
                tensor=rmsw, offset=0, ap=[[0, 32], [1, D]]))

            cs_st = []
            for s in range(2):
                cst = const.tile([32, D], F32, tag=f"cs{s}", name=f"cs{s}")
                nc.sync.dma_start(out=cst, in_=cs_k[32 * s:32 * s + 32, :])
                cs_st.append(cst)

            hf = []
            for s in range(2):
                h = const.tile([128, 1], F32, tag=f"hf{s}")
                nc.sync.dma_start(out=h, in_=bass.AP(
                    tensor=haloflag, offset=s, ap=[[0, 128], [1, 1]]))
                hf.append(h)

            for s in range(2):
                o = 132 * s
                gates = work.tile([128, 32, 8], F32, tag="gates")
                so_v = sc_old[:, o:o + 128].rearrange("p (g x) -> p g x", x=4)
                sn_v = sc_new[:, o + 4:o + 132].rearrange("p (g x) -> p g x", x=4)
                ko_v = kv_old[:, o:o + 128].rearrange("p (g x) -> p g x", x=4)
                kn_v = kv_new[:, o + 4:o + 132].rearrange("p (g x) -> p g x", x=4)
                nc.vector.tensor_add(gates[:, :, 0:4], so_v, ape_rep[:, :, 0:4])
                nc.vector.tensor_add(gates[:, :, 4:8], sn_v, ape_rep[:, :, 4:8])
                # first group's old slots += -1e30 when strip starts at t=0
                nc.vector.tensor_scalar(gates[:, 0, 0:4], gates[:, 0, 0:4],
                                        hf[s], None, op0=mybir.AluOpType.add)
                gmax = work.tile([128, 32], F32, tag="gmax")
                nc.vector.reduce_max(gmax, gates, axis=mybir.AxisListType.X)
                nc.vector.tensor_sub(gates, gates,
                                     gmax.to_broadcast([128, 32, 8]))
                ex = work.tile([128, 32, 8], F32, tag="ex")
                nc.scalar.activation(ex, gates, mybir.ActivationFunctionType.Exp)
                den = work.tile([128, 32], F32, tag="den")
                nc.vector.reduce_sum(den, ex, axis=mybir.AxisListType.X)
                rec = work.tile([128, 32], F32, tag="rec")
                nc.vector.reciprocal(rec, den)
                w8 = work.tile([128, 32, 8], F32, tag="w8")
                nc.vector.tensor_mul(w8, ex, rec.to_broadcast([128, 32, 8]))
                prod = work.tile([128, 32, 8], F32, tag="prod")
                nc.vector.tensor_mul(prod[:, :, 0:4], w8[:, :, 0:4], ko_v)
                nc.vector.tensor_mul(prod[:, :, 4:8], w8[:, :, 4:8], kn_v)
                comp = work.tile([128, 32], F32, tag="comp")
                nc.vector.reduce_sum(comp, prod, axis=mybir.AxisListType.X)

                cps = tpsB.tile([32, 128], F32, tag="ctp")
                tp(cps, comp)
                compT = work.tile([32, D], F32, tag="compT")
                nc.scalar.copy(compT, cps)

                # RMSNorm over d
                sq = work.tile([32, D], F32, tag="sq")
                nc.vector.tensor_mul(sq, compT, compT)
                ssum = work.tile([32, 1], F32, tag="ssum")
                nc.vector.reduce_sum(ssum, sq, axis=mybir.AxisListType.X)
                nc.vector.tensor_scalar(ssum, ssum, 1.0 / D, EPS,
                                        op0=mybir.AluOpType.mult,
                                        op1=mybir.AluOpType.add)
                rt = work.tile([32, 1], F32, tag="rt")
                nc.scalar.sqrt(rt, ssum)
                rs = work.tile([32, 1], F32, tag="rs")
                nc.vector.reciprocal(rs, rt)
                nc.vector.tensor_scalar(compT, compT, rs, None,
                                        op0=mybir.AluOpType.mult)
                nc.vector.tensor_mul(compT, compT, rms_rep)

                # RoPE at compressed positions (all tiles at base partition 0)
                co = cs_st[s][:, 0:64]
                si = cs_st[s][:, 64:128]
                x1 = compT[:, 0:64]
                x2 = compT[:, 64:128]
                tmp = work.tile([32, D], F32, tag="ktmp")
                kx = work.tile([32, D], F32, tag="kx")
                nc.vector.tensor_mul(kx[:, 0:64], x1, co)
                nc.vector.tensor_mul(tmp[:, 0:64], x2, si)
                nc.vector.tensor_sub(kx[:, 0:64], kx[:, 0:64], tmp[:, 0:64])
                nc.vector.tensor_mul(kx[:, 64:128], x2, co)
                nc.vector.tensor_mul(tmp[:, 64:128], x1, si)
                nc.vector.tensor_add(kx[:, 64:128], kx[:, 64:128],
                                     tmp[:, 64:128])
                nc.sync.dma_start(out=k_loc[32 * s:32 * s + 32, :], in_=kx)

    nc.finalize()
    return nc


# --------------------------------------------------------------------------
# launch 2: q GEMM + RoPE + qk + score assembly + mask + top-k
# --------------------------------------------------------------------------
def _build_l2():
    nc = bacc.Bacc()
    qr_sh = nc.declare_dram_parameter("qr_sh", [256, QR_DIM], F32, isOutput=False)
    wq = nc.declare_dram_parameter("wq", [H, 128, 12, 128], F32, isOutput=False)
    cs_own = nc.declare_dram_parameter("cs_own", [256, D], F32, isOutput=False)
    k_full = nc.declare_dram_parameter("k_full", [C, D], F32, isOutput=False)
    wts_own = nc.declare_dram_parameter("wts_own", [256, H], F32, isOutput=False)
    posm3 = nc.declare_dram_parameter("posm3", [256], F32, isOutput=False)
    out_idx = nc.declare_dram_parameter("out_idx", [256, TOPK], I32, isOutput=True)

    WIDTHS = (256, 512)  # candidate widths for (low tile j<=7, high tile)

    with tile.TileContext(nc) as tc, ExitStack() as ctx:
        const = ctx.enter_context(tc.tile_pool(name="const", bufs=1))
        work = ctx.enter_context(tc.tile_pool(name="work", bufs=2))
        tk = ctx.enter_context(tc.tile_pool(name="tk", bufs=2))

        ident = const.tile([128, 128], F32)
        make_identity(nc, ident)

        def tp(ps_out, in_sb):
            p = in_sb.shape[0]
            nc.tensor.transpose(ps_out, in_sb, ident[:p, :p])

        qrT = const.tile([128, 12, 256], F32)
        csT = const.tile([128, 256], F32)
        kT = const.tile([128, C], F32)
        with tc.tile_pool(name="stg", bufs=2) as stg, \
             tc.tile_pool(name="tps", bufs=2, space="PSUM") as tps:
            for tt in range(2):
                stage = stg.tile([128, QR_DIM], F32, tag="qstage")
                nc.sync.dma_start(out=stage,
                                  in_=qr_sh[tt * 128:(tt + 1) * 128, :])
                for kg in range(3):
                    ps = tps.tile([128, 512], F32, tag="tp")
                    for u in range(4):
                        kc = kg * 4 + u
                        tp(ps[:, u * 128:(u + 1) * 128],
                           stage[:, kc * 128:(kc + 1) * 128])
                    nc.scalar.copy(
                        qrT[:, kg * 4:kg * 4 + 4, tt * 128:(tt + 1) * 128],
                        ps.rearrange("p (u x) -> p u x", x=128))
            for tt in range(2):
                stage = stg.tile([128, D], F32, tag="cstage")
                nc.sync.dma_start(out=stage,
                                  in_=cs_own[tt * 128:(tt + 1) * 128, :])
                ps = tps.tile([128, 512], F32, tag="tp")
                tp(ps[:, :128], stage)
                nc.scalar.copy(csT[:, tt * 128:(tt + 1) * 128], ps[:, :128])
            kstage = const.tile([128, 4, D], F32)
            nc.sync.dma_start(out=kstage,
                              in_=k_full[:].rearrange("(a p) d -> p a d", p=128))
            for a in range(4):
                ps = tps.tile([128, 512], F32, tag="tp")
                tp(ps[:, :128], kstage[:, a, :])
                nc.scalar.copy(kT[:, a * 128:(a + 1) * 128], ps[:, :128])

        # cc = [cos;cos], ss = [-sin;sin] (partition moves via DMA only)
        cc = const.tile([128, 256], F32)
        ss = const.tile([128, 256], F32)
        nc.sync.dma_start(out=cc[0:64, :], in_=csT[0:64, :])
        nc.sync.dma_start(out=cc[64:128, :], in_=csT[0:64, :])
        nc.sync.dma_start(out=ss[0:64, :], in_=csT[64:128, :])
        nc.sync.dma_start(out=ss[64:128, :], in_=csT[64:128, :])
        nc.vector.tensor_scalar(ss[0:64, :], ss[0:64, :], -1.0, None,
                                op0=mybir.AluOpType.mult)

        wts_sb, pos_sb = [], []
        for tt in range(2):
            w = const.tile([128, H], F32, tag=f"wts{tt}")
            nc.sync.dma_start(out=w, in_=wts_own[tt * 128:(tt + 1) * 128, :])
            wts_sb.append(w)
            p = const.tile([128, 1], F32, tag=f"pos{tt}")
            nc.sync.dma_start(out=p, in_=posm3[tt * 128:(tt + 1) * 128])
            pos_sb.append(p)

        c4p = const.tile([128, C], F32)
        nc.gpsimd.iota(c4p, pattern=[[4, C]], base=0, channel_multiplier=0,
                       allow_small_or_imprecise_dtypes=True)
        c4f = const.tile([128, C], F32)
        nc.vector.tensor_scalar(c4f, c4p, -1.0, None, op0=mybir.AluOpType.mult)
        negs = const.tile([128, C], F32)
        nc.vector.memset(negs, NEG)
        neg1 = const.tile([128, TOPK], I32)
        nc.vector.memset(neg1, -1)

        acc = [const.tile([128, C], F32, tag=f"acc{tt}", name=f"acc{tt}") for tt in range(2)]

        def emit_topk(tt):
            W = WIDTHS[tt]
            cmp = work.tile([128, C], F32, tag="cmp", name="cmp")
            nc.vector.tensor_scalar(cmp[:, :W], c4f[:, :W], pos_sb[tt], None,
                                    op0=mybir.AluOpType.add)
            mbit = work.tile([128, C], U32, tag="mbit", name="mbit")
            nc.vector.tensor_scalar(mbit[:, :W], cmp[:, :W], 0.0, None,
                                    op0=mybir.AluOpType.is_lt)
            nc.vector.copy_predicated(acc[tt][:, :W], mbit[:, :W],
                                      negs[:, :W])

            idx = tk.tile([128, TOPK], U32, tag="idx", name="idx")
            vals = acc[tt]
            for it in range(32):
                mx = tk.tile([128, 8], F32, tag="mx", name="mx")
                nc.vector.max(out=mx, in_=vals[:, :W])
                nc.vector.max_index(out=idx[:, it * 8:(it + 1) * 8],
                                    in_max=mx, in_values=vals[:, :W])
                nc.vector.match_replace(out=vals[:, :W], in_to_replace=mx,
                                        in_values=vals[:, :W], imm_value=NEG)

            idx32 = tk.tile([128, TOPK], I32, tag="idx32", name="idx32")
            nc.vector.tensor_copy(idx32, idx)
            rmp = work.tile([128, TOPK], F32, tag="rmp", name="rmp")
            nc.vector.tensor_scalar(rmp, c4f[:, :TOPK], pos_sb[tt], None,
                                    op0=mybir.AluOpType.add)
            rbit = work.tile([128, TOPK], U32, tag="rbit", name="rbit")
            nc.vector.tensor_scalar(rbit, rmp, 0.0, None,
                                    op0=mybir.AluOpType.is_lt)
            nc.vector.copy_predicated(idx32, rbit, neg1)
            nc.sync.dma_start(out=out_idx[tt * 128:(tt + 1) * 128, :],
                              in_=idx32)

        qro_keep = ctx.enter_context(tc.tile_pool(name="qro_keep", bufs=H))
        qros = []
        with tc.tile_pool(name="wqp", bufs=3) as wqp, \
             tc.tile_pool(name="qro_p", bufs=2) as qro_p, \
             tc.tile_pool(name="qps", bufs=2, space="PSUM") as qps, \
             tc.tile_pool(name="qkps", bufs=2, space="PSUM") as qkps:
            for m in range(H):
                wqt = wqp.tile([128, 12, 128], F32, tag="wq")
                nc.sync.dma_start(out=wqt, in_=wq[m])
                ps_q = qps.tile([128, 256], F32, tag="qps")
                for kc in range(12):
                    nc.tensor.matmul(ps_q, wqt[:, kc, :], qrT[:, kc, :],
                                     start=(kc == 0), stop=(kc == 11))
                q_sb = qro_p.tile([128, 256], F32, tag="q_sb")
                nc.scalar.copy(q_sb, ps_q)
                q_sw = qro_p.tile([128, 256], F32, tag="q_sw")
                nc.sync.dma_start(out=q_sw[0:64, :], in_=q_sb[64:128, :])
                nc.sync.dma_start(out=q_sw[64:128, :], in_=q_sb[0:64, :])
                qro = qro_keep.tile([128, 256], F32, tag="qro")
                tmp = qro_p.tile([128, 256], F32, tag="qtmp")
                nc.vector.tensor_mul(qro, q_sb, cc)
                nc.vector.tensor_mul(tmp, q_sw, ss)
                nc.vector.tensor_add(qro, qro, tmp)
                qros.append(qro)

            def emit_qk(tt, m, qro):
                Wtt = WIDTHS[tt]
                ps_qk = qkps.tile([128, Wtt], F32, tag=f"qkps{tt}",
                                  name=f"ps_qk{tt}")
                nc.tensor.matmul(ps_qk, qro[:, tt * 128:(tt + 1) * 128],
                                 kT[:, :Wtt], start=True, stop=True)
                if m == 0:
                    nc.vector.tensor_scalar(
                        acc[tt][:, :Wtt], ps_qk, 0.0, wts_sb[tt][:, m:m + 1],
                        op0=mybir.AluOpType.max, op1=mybir.AluOpType.mult)
                else:
                    rl = work.tile([128, Wtt], F32, tag=f"rl{tt}",
                                   name=f"rl{tt}")
                    nc.vector.tensor_scalar(
                        rl, ps_qk, 0.0, wts_sb[tt][:, m:m + 1],
                        op0=mybir.AluOpType.max, op1=mybir.AluOpType.mult)
                    nc.vector.tensor_add(acc[tt][:, :Wtt],
                                         acc[tt][:, :Wtt], rl)

            for m in range(H):
                emit_qk(0, m, qros[m])
            emit_topk(0)
            for m in range(H):
                emit_qk(1, m, qros[m])
            emit_topk(1)

    nc.finalize()
    return nc


def _get(name):
    if name not in _cache:
        _cache[name] = _build_l1() if name == "l1" else _build_l2()
    return _cache[name]


def kernel(hidden_states, qr, positions, W_fused, Wq, Wproj, ape, rms_weight,
           cos_sin_cache, _timing=None):
    hidden_states = np.asarray(hidden_states, np.float32)
    qr = np.asarray(qr, np.float32)
    positions = np.asarray(positions, np.int32)
    W_fused = np.asarray(W_fused, np.float32)
    Wq = np.asarray(Wq, np.float32)
    Wproj = np.asarray(Wproj, np.float32)
    ape = np.asarray(ape, np.float32)
    rms_weight = np.asarray(rms_weight, np.float32)
    cos_sin_cache = np.asarray(cos_sin_cache, np.float32)

    wcomb = np.ascontiguousarray(
        np.concatenate([W_fused.T, Wproj.T], axis=1))          # [7168, 576]
    wq_pre = np.ascontiguousarray(
        Wq.reshape(H, 128, 12, 128).transpose(0, 3, 2, 1))     # [m, kk, kc, mm]

    cores = list(range(NC))
    trace = _timing is not None

    in1 = []
    for i in cores:
        rows = []
        for j in PAIRS[i]:
            lo = 128 * j - 4
            if lo < 0:
                blk = np.zeros((132, HID), np.float32)
                blk[4:] = hidden_states[0:128 * j + 128]
            else:
                blk = hidden_states[lo:128 * j + 128]
            rows.append(blk)
        hidden_halo = np.ascontiguousarray(np.concatenate(rows, axis=0))
        cs_rows = np.concatenate(
            [cos_sin_cache[32 * j:32 * j + 32] for j in PAIRS[i]], axis=0)
        hfl = np.array([NEG if j == 0 else 0.0 for j in PAIRS[i]], np.float32)
        in1.append({
            "hidden": hidden_halo, "wcomb": wcomb, "ape": ape,
            "rmsw": rms_weight, "cs_k": np.ascontiguousarray(cs_rows),
            "haloflag": hfl,
        })
    r1 = run_bass_kernel_spmd(_get("l1"), in1, cores, trace=trace,
                              trace_cores=cores if trace else None)

    k_full = np.zeros((C, D), np.float32)
    wts = {}
    for i in cores:
        kl = r1.results[i]["k_loc"]
        for s, j in enumerate(PAIRS[i]):
            k_full[32 * j:32 * j + 32] = kl[32 * s:32 * s + 32]
        wts[i] = r1.results[i]["wts_own"]

    in2 = []
    for i in cores:
        sel = np.concatenate(
            [np.arange(128 * j, 128 * j + 128) for j in PAIRS[i]])
        in2.append({
            "qr_sh": np.ascontiguousarray(qr[sel]),
            "wq": wq_pre,
            "cs_own": np.ascontiguousarray(cos_sin_cache[positions[sel]]),
            "k_full": k_full,
            "wts_own": wts[i],
            "posm3": (positions[sel] - 3).astype(np.float32),
        })
    r2 = run_bass_kernel_spmd(_get("l2"), in2, cores, trace=trace,
                              trace_cores=cores if trace else None)

    out = np.empty((T, TOPK), np.int32)
    for i in cores:
        oi = r2.results[i]["out_idx"]
        for s, j in enumerate(PAIRS[i]):
            out[128 * j:128 * j + 128] = oi[128 * s:128 * s + 128]

    if _timing is not None:
        _timing["l1"] = r1
        _timing["l2"] = r2
    return out
